# revision 34
# baseline (speedup 1.0000x reference)
"""Trainium2 Bass kernel for BasicMoE — gate-centered fp8 formulation.

Reference (N=8192 tokens, D=1024 in, O=1024 out, E=8 experts):
    gates = softmax(x @ Wg + bg)                        # [N, E]
    out   = sum_e gates[:, e] * (x @ We[e] + be[e])     # [N, O]

Rewrite with d_e = g_e - 1/8 and Wavg = mean_e We (sum_e g_e = 1):
    out = x @ Wavg  +  sum_e (d_e * x) @ We[e]  +  g @ be

The mean path x@Wavg carries most of the output mass and runs in bf16.
The 8 expert matmuls run in fp8 e4m3 with DoubleRow perf mode (2 MACs
per PE cell per cycle); their operands only carry the gate-DEVIATION-
weighted activations u_e = d_e * x, so fp8 quantization noise lands on
~half the output mass. Measured end-to-end rel err ~1.72e-2 (< 2e-2).

fp8 scaling (e4m3 min normal is 2^-6; We and u would be subnormal):
    u8  = 32 * d_e * x      (|.| <~ 150 < 240)
    We8 = 512 * We          (|.| <= 16)
    Wavg_sc = 16384 * Wavg, be_sc = 16384 * be  (bf16)
PSUM accumulates 16384*out; the output copy descales by 2^-14 (exact).

Data-parallel over tokens: each core takes 1024 tokens + replicated
weights. Per-core schedule (all engines overlapped):
  - xt arrives as per-k-chunk DMAs on both HWDGE rings; gating matmuls
    consume chunks in arrival order (k emission order 0,4,1,5,...).
  - gating is TRANSPOSED: lhsT=Wg chunk [128,8] (tiny LDWEIGHTS),
    rhs=xt chunk -> zT [E=8 partitions, tokens] in PSUM. Softmax:
    no max-subtraction (|logits| < ~4, exp is safe in f32);
    ex = ACT exp(zT + bgT); per-token sums via a tiny ones[8,8] matmul
    (cross-partition sum on PE); reciprocal+scale on DVE per t-half so
    the first half's gate deviations come out early.
  - u_e tiles are built per (expert, half): gpsimd moves dT32 row e to
    partition 0 (row 0 needs no move), partition_broadcasts it, DVE
    multiplies with xt -> fp8. Generous dbc/u8 buffer counts keep the
    gpsimd->DVE->PE pipeline from self-pacing.
  - 16 dummy matmuls on memset data pre-warm the PE HAM clock gate
    (1.2 -> 2.4 GHz) during the initial DMA wait.
  - main loop: two t-halves of 8 PSUM banks (4 t x 2 j). Per half:
    k-major Wavg bf16 MMs open the banks (no gate dependency), bias MMs
    (g @ be, K=8), then per-expert fp8 DoubleRow MMs (2 MMs per
    LDWEIGHTS, moving free dim 1024 fp8 -> 512-col PSUM writes).
    Last expert runs t-major so banks close early; they are staged to
    SBUF with the 2^-14 descale (half A on ACT, half B on DVE to dodge
    ACT FIFO head-of-line waits) and DMA out while compute continues.

Host side: expert weights are mean-centered before quantization
(sum_e d_e = 0 makes this exact) for ~2% less fp8 noise.
Typical HW exec: ~172-175us (vs 258us bf16 baseline), rel err 1.69e-2.
"""

import numpy as np
import ml_dtypes

N_TOKENS = 8192
D = 1024   # in dim
O = 1024   # out dim
E = 8      # experts
NCORES = 8
NLOC = N_TOKENS // NCORES   # 1024 tokens per core
KT = D // 128               # 8 k-chunks
TT = NLOC // 128            # 8 token chunks
JT = O // 512               # 2 out chunks of 512
HALF = NLOC // 2            # 512 tokens per half

SU = 32.0                   # u scale
SW = 512.0                  # We scale
SOUT = SU * SW              # 16384 = total PSUM scale

BF16 = ml_dtypes.bfloat16
F8 = ml_dtypes.float8_e4m3  # TRN fp8e4: max +-240, matches ml_dtypes ieee e4m3

_CACHE = {}


def _build():
    import concourse.bass as bass
    import concourse.mybir as mybir
    import concourse.tile as tile
    from concourse import bacc

    dt = mybir.dt
    f32 = dt.float32
    bf16 = dt.bfloat16
    fp8 = dt.float8e4
    Alu = mybir.AluOpType
    DR = mybir.MatmulPerfMode.DoubleRow

    nc = bacc.Bacc(
        "TRN2",
        target_bir_lowering=False,
        debug=False,
        enable_asserts=False,
        num_devices=NCORES,
    )

    xt_d = nc.dram_tensor("xt", [128, KT * NLOC], bf16, kind="ExternalInput").ap()
    we_d = nc.dram_tensor("We8", [E, 128, KT * O], fp8, kind="ExternalInput").ap()
    wavg_d = nc.dram_tensor("Wavg", [128, KT * O], bf16, kind="ExternalInput").ap()
    wg_d = nc.dram_tensor("Wgp", [128, KT * E], bf16, kind="ExternalInput").ap()
    bgt_d = nc.dram_tensor("bgT", [E, 1], f32, kind="ExternalInput").ap()
    bet_d = nc.dram_tensor("beT", [E, O], bf16, kind="ExternalInput").ap()
    out_d = nc.dram_tensor("out", [NLOC, O], f32, kind="ExternalOutput").ap()

    with tile.TileContext(nc) as tc:
        with (
            tc.tile_pool(name="const", bufs=1) as cpool,
            tc.tile_pool(name="xp", bufs=1) as xpool,
            tc.tile_pool(name="wavgp", bufs=1) as wavgpool,
            tc.tile_pool(name="wp", bufs=E) as wpool,
            tc.tile_pool(name="up", bufs=6) as upool,
            tc.tile_pool(name="dbp", bufs=1) as dbpool,
            tc.tile_pool(name="gp", bufs=1) as gpool,
            tc.tile_pool(name="op", bufs=8) as opool,
        ):
            # Gating weights first on the fast rings (gating MMs are the
            # kernel's warm-up work); bet on the SWDGE (gpsimd) queue.
            wg_sb = cpool.tile([128, KT, E], bf16)
            nc.sync.dma_start(wg_sb[:], wg_d.rearrange("p (k e) -> p k e", k=KT))
            bgt_sb = cpool.tile([E, 1], f32)
            nc.gpsimd.dma_start(bgt_sb[:], bgt_d)
            bet_sb = cpool.tile([E, O], bf16)
            nc.gpsimd.dma_start(bet_sb[:], bet_d)

            # xt as per-k-chunk DMAs so gating can start on the first
            # 256KB: ring A gets k0-3, ring B k4-7.
            xt = xpool.tile([128, KT, NLOC], bf16)
            xt_v = xt_d.rearrange("p (k n) -> p k n", k=KT)
            for k in range(KT // 2):
                nc.sync.dma_start(xt[:, k, :], xt_v[:, k, :])
            for k in range(KT // 2, KT):
                nc.scalar.dma_start(xt[:, k, :], xt_v[:, k, :])

            # Wavg per-k chunks split across both rings (k4-7 ride ring A
            # ahead of We8; k0-3 ring B behind xt k4-7) so all of Wavg
            # lands while the gating matmuls still run.
            wavg = wavgpool.tile([128, KT, O], bf16)
            wavg_v = wavg_d.rearrange("p (k o) -> p k o", k=KT)
            for k in range(KT // 2, KT):
                nc.sync.dma_start(wavg[:, k, :], wavg_v[:, k, :])
            for k in range(KT // 2):
                nc.scalar.dma_start(wavg[:, k, :], wavg_v[:, k, :])

            # Expert fp8 weights: one contiguous DMA per expert (8KB rows)
            # on ring A behind xt k0-3.
            we_tiles = []
            for e in range(E):
                we_sb = wpool.tile([128, KT, O], fp8, tag="we", name=f"we{e}")
                nc.sync.dma_start(
                    we_sb[:], we_d[e].rearrange("p (k o) -> p k o", k=KT)
                )
                we_tiles.append(we_sb)

            # ---- Gating (transposed): zT[e, n] = sum_k Wg[k,e] x[n,k] ----
            ex_f = gpool.tile([E, NLOC], f32)     # exp(z + bg)
            gT = gpool.tile([E, NLOC], bf16)      # gates, bias-MM lhsT
            dT32 = gpool.tile([E, NLOC], bf16)    # 32*g - 4, broadcast src
            sm = gpool.tile([E, NLOC], f32)       # per-token sums (all rows)
            rcp = gpool.tile([E, NLOC], f32)      # 1/sum

            # Main PSUM pool opened early: the gating zT tiles draw from
            # it too, so there is no pool stacking — the two acc banks
            # that later land on the zT slots just wait for the ACT exp
            # reads, which complete before their first matmul anyway.
            psM = tc.alloc_tile_pool(name="psM", bufs=8, space="PSUM")

            korder = [0, 4, 1, 5, 2, 6, 3, 7]
            zps = [psM.tile([E, HALF], f32, tag="acc", name=f"zg{h}",
                            padded_shape=[128, 512])
                   for h in range(2)]

            # Pre-warm the PE's HAM clock gate during the initial DMA
            # wait: ~3.4us of dummy matmuls on memset data lift the PE
            # from 1.2 to 2.4 GHz before the real gating work arrives.
            # Results land in zps[0] and are discarded by the real
            # gating matmul's start=True.
            warm = cpool.tile([128, 512], bf16)
            nc.gpsimd.memset(warm[:], 0.0)
            for w in range(10):
                nc.tensor.matmul(
                    zps[0][:], warm[:, :E], warm[:], start=True, stop=True,
                )
            # h-separated so half 0's logits close early and its
            # softmax tail overlaps half 1's gating matmuls.
            for h in range(2):
                for i, k in enumerate(korder):
                    nc.tensor.matmul(
                        zps[h][:],
                        wg_sb[:, k, :],
                        xt[:, k, h * HALF : (h + 1) * HALF],
                        start=(i == 0),
                        stop=(i == KT - 1),
                    )
                # exp(z + bg); no max-sub (|z| < ~4, f32 exp is safe)
                nc.scalar.activation(
                    ex_f[:, h * HALF : (h + 1) * HALF],
                    zps[h][:],
                    mybir.ActivationFunctionType.Exp,
                    bias=bgt_sb[:],
                )

            def softmax_tail(h):
                """sums (gpsimd cross-partition add), 1/sum, g, 32g-4."""
                hs = slice(h * HALF, (h + 1) * HALF)
                nc.gpsimd.partition_all_reduce(
                    sm[:, hs], ex_f[:, hs], channels=E,
                    reduce_op=bass.bass_isa.ReduceOp.add,
                )
                nc.vector.reciprocal(rcp[:, hs], sm[:, hs])
                nc.vector.tensor_mul(gT[:, hs], ex_f[:, hs], rcp[:, hs])
                nc.vector.tensor_scalar(
                    dT32[:, hs], gT[:, hs], SU, -SU / 8.0,
                    op0=Alu.mult, op1=Alu.add,
                )

            # ---- Main: two halves of (4 t-chunks x 2 j) PSUM banks ------
            def emit_half(half):
                t0 = half * (TT // 2)
                nsl = slice(half * HALF, (half + 1) * HALF)

                banks = {}
                for ti in range(TT // 2):
                    for j in range(JT):
                        banks[(ti, j)] = psM.tile(
                            [128, 512], f32, tag="acc",
                            name=f"acc{half}_{ti}_{j}")

                # Wavg bf16 path opens each bank (no gate dependency);
                # k-major in DMA-arrival order (rings deliver k4/k0/k5/...).
                ks = [4, 0, 5, 1, 6, 2, 7, 3] if half == 0 else list(range(KT))
                for ki, k in enumerate(ks):
                    for ti in range(TT // 2):
                        t = t0 + ti
                        lhs = xt[:, k, t * 128 : (t + 1) * 128]
                        for j in range(JT):
                            nc.tensor.matmul(
                                banks[(ti, j)][:],
                                lhs,
                                wavg[:, k, j * 512 : (j + 1) * 512],
                                start=(ki == 0),
                                stop=False,
                            )

                # u8 tiles for this half (DVE), expert-by-expert.
                # partition_broadcast reads partition 0 only: expert 0's
                # row is already there (no staging); rows 1-7 are staged
                # down by tiny SBUF->SBUF DMAs, all issued right after
                # broadcast 0 so their ~6us SWDGE latencies overlap.
                dmovs = {
                    e: dbpool.tile([1, HALF], bf16, tag=f"dmv{e}",
                                   name=f"dm{half}_{e}")
                    for e in range(1, E)
                }
                u_tiles = []
                for e in range(E):
                    src = dT32[0:1, nsl] if e == 0 else dmovs[e][0:1, :]
                    dbc = dbpool.tile([128, HALF], bf16, tag="dbc", bufs=16,
                                      name=f"db{half}_{e}")
                    nc.gpsimd.partition_broadcast(dbc[:], src)
                    if e == 0:
                        for ee in range(1, E):
                            nc.gpsimd.dma_start(
                                dmovs[ee][:], dT32[ee : ee + 1, nsl]
                            )
                    u8 = upool.tile([128, KT, HALF], fp8, tag="u8",
                                    name=f"u{half}_{e}")
                    for k in range(KT):
                        nc.vector.tensor_mul(u8[:, k, :], xt[:, k, nsl], dbc[:])
                    u_tiles.append(u8)

                # Bias MMs (g @ be, K=8) after the Wavg block: gT is ready
                # well before the block ends.
                for ti in range(TT // 2):
                    t = t0 + ti
                    for j in range(JT):
                        nc.tensor.matmul(
                            banks[(ti, j)][:],
                            gT[:, t * 128 : (t + 1) * 128],
                            bet_sb[:, j * 512 : (j + 1) * 512],
                            start=False,
                            stop=False,
                        )

                # Expert fp8 DoubleRow MMs; 2 MMs per LDWEIGHTS.
                # Last expert t-major so banks close early and drain.
                for e in range(E):
                    u8 = u_tiles[e]
                    we_sb = we_tiles[e]
                    last = e == E - 1
                    if not last:
                        for kp in range(KT // 2):
                            for ti in range(TT // 2):
                                lhs = u8[:, 2 * kp : 2 * kp + 2,
                                         ti * 128 : (ti + 1) * 128]
                                for j in range(JT):
                                    nc.tensor.matmul(
                                        banks[(ti, j)][:],
                                        lhs,
                                        we_sb[:, 2 * kp : 2 * kp + 2,
                                              j * 512 : (j + 1) * 512],
                                        start=False,
                                        stop=False,
                                        perf_mode=DR,
                                    )
                    else:
                        for ti in range(TT // 2):
                            t = t0 + ti
                            for kp in range(KT // 2):
                                lhs = u8[:, 2 * kp : 2 * kp + 2,
                                         ti * 128 : (ti + 1) * 128]
                                for j in range(JT):
                                    nc.tensor.matmul(
                                        banks[(ti, j)][:],
                                        lhs,
                                        we_sb[:, 2 * kp : 2 * kp + 2,
                                              j * 512 : (j + 1) * 512],
                                        start=False,
                                        stop=(kp == KT // 2 - 1),
                                        perf_mode=DR,
                                    )
                            for j in range(JT):
                                # PSUM is not DMA-readable: stage with the
                                # 2^-14 descale folded in, then DMA out.
                                # Half A on ACT; half B on DVE (free by
                                # then) so the final drains are not stuck
                                # behind ACT FIFO head-of-line waits.
                                stg = opool.tile([128, 512], f32, tag="stg",
                                                 name=f"st{half}_{ti}_{j}")
                                # The very last bank is on the kernel's
                                # critical tail: drain it in 256-col
                                # pieces so the final DMA issues sooner.
                                last_bank = half == 1 and ti == TT // 2 - 1
                                nsplit = 2 if last_bank else 1
                                for q in range(nsplit):
                                    c0 = q * (512 // nsplit)
                                    c1 = (q + 1) * (512 // nsplit)
                                    if half == 0:
                                        nc.scalar.activation(
                                            stg[:, c0:c1],
                                            banks[(ti, j)][:, c0:c1],
                                            mybir.ActivationFunctionType.Copy,
                                            scale=1.0 / SOUT,
                                        )
                                    else:
                                        nc.vector.tensor_scalar(
                                            stg[:, c0:c1],
                                            banks[(ti, j)][:, c0:c1],
                                            1.0 / SOUT, None, op0=Alu.mult,
                                        )
                                    # Alternate rings so the final two
                                    # bank drains issue concurrently.
                                    ring = nc.sync if j == 0 else nc.scalar
                                    ring.dma_start(
                                        out_d[t * 128 : (t + 1) * 128,
                                              j * 512 + c0 : j * 512 + c1],
                                        stg[:, c0:c1],
                                    )

            softmax_tail(0)
            emit_half(0)
            softmax_tail(1)
            emit_half(1)
            psM.release()

    nc.compile()
    return nc


def _get_nc():
    if "nc" not in _CACHE:
        _CACHE["nc"] = _build()
    return _CACHE["nc"]


def _pack_inputs(x, We, be, Wg, bg):
    """Host-side packing: shard tokens, pre-transpose, quantize."""
    x = np.asarray(x, dtype=np.float32)
    We = np.asarray(We, dtype=np.float32)
    be = np.asarray(be, dtype=np.float32)
    Wg = np.asarray(Wg, dtype=np.float32)
    bg = np.asarray(bg, dtype=np.float32)

    # Expert weights are CENTERED before fp8 quantization: with
    # sum_e d_e = 0, sum_e d_e*(x@We) == sum_e d_e*(x@(We-Wavg)), and
    # the centered weights have slightly smaller variance -> less fp8
    # quantization noise. We8[e][p, k*O + o] = 512*(We-Wavg)[e, k*128+p, o]
    wcent = We - We.mean(0, keepdims=True)
    we8 = np.ascontiguousarray(
        (wcent * SW).reshape(E, KT, 128, O).transpose(0, 2, 1, 3).reshape(E, 128, KT * O)
    ).astype(F8)
    # Wavg[p, k*O + o] = 16384 * mean_e We  (bf16)
    wavg = np.ascontiguousarray(
        (We.mean(0) * SOUT).reshape(KT, 128, O).transpose(1, 0, 2).reshape(128, KT * O)
    ).astype(BF16)
    wg_p = np.ascontiguousarray(
        Wg.reshape(KT, 128, E).transpose(1, 0, 2).reshape(128, KT * E)
    ).astype(BF16)
    bgt = bg.reshape(E, 1).astype(np.float32)
    bet = (be * SOUT).astype(BF16)

    in_maps = []
    for i in range(NCORES):
        xs = x[i * NLOC : (i + 1) * NLOC]          # [NLOC, D]
        xt = np.ascontiguousarray(
            xs.T.reshape(KT, 128, NLOC).transpose(1, 0, 2).reshape(128, KT * NLOC)
        ).astype(BF16)
        in_maps.append(
            {"xt": xt, "We8": we8, "Wavg": wavg, "Wgp": wg_p,
             "bgT": bgt, "beT": bet}
        )
    return in_maps


def _run(inputs, trace=False):
    from concourse.bass_utils import run_bass_kernel_spmd

    nc = _get_nc()
    in_maps = _pack_inputs(**inputs)
    res = run_bass_kernel_spmd(
        nc, in_maps, core_ids=list(range(NCORES)), trace=trace
    )
    y = np.concatenate(
        [res.results[i]["out"] for i in range(NCORES)], axis=0
    ).astype(np.float32)
    return y, res


def kernel(**inputs):
    y, _ = _run(inputs, trace=False)
    return y
